# revision 8
# baseline (speedup 1.0000x reference)
"""TRN2 Bass kernel for nn_AddMessagePasser (GNN message passing).

h = relu((emb[edge_type] + node_feat[src_idx] + edge_feat) @ W.T + b)

Strategy (8 NeuronCores, SPMD, data-parallel over edges):
- Edges padded to 8 equal shards; node_feat / emb / W / b replicated.
- Per core, edges are sorted by src%4 into 4 groups (padded to 512-edge
  blocks) so node-feature gathers can use int16 indices into the subtable
  node_feat[k::4] (25000 rows < 32768) via SWDGE dma_gather.
- All gathers use dma_gather(transpose=True): gathered rows land in
  transposed [d, e] layout [128, 4, 512] bf16, which feeds the matmul lhsT
  directly (no on-chip transposes). Gathers spread over 4 SWDGE queues.
- edge_feat is host-pre-transposed into the same layout; plain HWDGE DMA.
- msgT = nfT + efT + embT (2 DVE adds, bf16).
- Per 128-edge tile: 4 matmuls accumulate psum[128e, 512f] (bf16 x bf16,
  fp32 accumulate); DVE adds bias; ACT applies ReLU; one 1MB store/block.
"""
import numpy as np
import ml_dtypes

import concourse.bacc as bacc
import concourse.mybir as mybir
import concourse.tile as tile
from concourse.masks import make_identity
from concourse import bass_utils

BF16 = mybir.dt.bfloat16
F32 = mybir.dt.float32
I16 = mybir.dt.int16

P = 128
D = 512
Fdim = 512
BLK = 512
N_NODES = 100000
N_RELS = 500
N_EDGES = 156250
N_CORES = 8
bf = ml_dtypes.bfloat16


def build_module(nb):
    """nb[k] = number of 512-edge blocks for src%4 group k (shared by all cores)."""
    NB = sum(nb)
    nc = bacc.Bacc("TRN2", debug=False, num_devices=1, num_swdge_queues=4)

    nfk = [nc.dram_tensor(f"nf{k}", [N_NODES // 4, D], BF16, kind="ExternalInput")
           for k in range(4)]
    eft = nc.dram_tensor("eft", [NB, P, 4, BLK], BF16, kind="ExternalInput")
    emb = nc.dram_tensor("emb", [N_RELS, D], BF16, kind="ExternalInput")
    src16 = nc.dram_tensor("src16", [P, NB * 32], I16, kind="ExternalInput")
    et16 = nc.dram_tensor("et16", [P, NB * 32], I16, kind="ExternalInput")
    wt = nc.dram_tensor("wt", [4, P, Fdim], BF16, kind="ExternalInput")
    biasf = nc.dram_tensor("biasf", [P, Fdim], F32, kind="ExternalInput")
    out = nc.dram_tensor("out", [NB, P, 4, Fdim], F32, kind="ExternalOutput")

    qctr = [0]

    def nextq():
        q = qctr[0] % 4
        qctr[0] += 1
        return q

    with tile.TileContext(nc) as tc:
        with (
            tc.tile_pool(name="const", bufs=1) as const_pool,
            tc.tile_pool(name="io", bufs=3) as io_pool,
            tc.tile_pool(name="msgp", bufs=4) as msg_pool,
            tc.tile_pool(name="hout", bufs=3) as h_pool,
            tc.tile_pool(name="htmp", bufs=6) as ht_pool,
            tc.tile_pool(name="mt2", bufs=6) as mt2_pool,
            tc.tile_pool(name="pst", bufs=3, space="PSUM") as psT_pool,
            tc.tile_pool(name="pso", bufs=5, space="PSUM") as psO_pool,
        ):
            src_sb = const_pool.tile([P, NB * 32], I16)
            nc.sync.dma_start(out=src_sb[:], in_=src16[:])
            et_sb = const_pool.tile([P, NB * 32], I16)
            nc.sync.dma_start(out=et_sb[:], in_=et16[:])
            wt_sb = const_pool.tile([P, 4, Fdim], BF16)
            nc.scalar.dma_start(out=wt_sb[:], in_=wt[:].rearrange("c p f -> p c f"))
            bias_sb = const_pool.tile([P, Fdim], F32)
            nc.scalar.dma_start(out=bias_sb[:], in_=biasf[:])
            ident_sb = const_pool.tile([P, P], BF16)
            make_identity(nc, ident_sb[:])

            b = 0
            for k in range(4):
                for _ in range(nb[k]):
                    nf_t = io_pool.tile([P, 4, BLK], BF16, tag="nf")
                    nc.gpsimd.dma_gather(
                        out_ap=nf_t[:], in_ap=nfk[k][:],
                        idxs_ap=src_sb[:, b * 32:(b + 1) * 32],
                        num_idxs=BLK, num_idxs_reg=BLK, elem_size=D,
                        transpose=True, queue_num=nextq(),
                    )
                    em_t = io_pool.tile([P, 4, D], BF16, tag="em")
                    nc.gpsimd.dma_gather(
                        out_ap=em_t[:], in_ap=emb[:],
                        idxs_ap=et_sb[:, b * 32:(b + 1) * 32],
                        num_idxs=BLK, num_idxs_reg=BLK, elem_size=D,
                        transpose=False, queue_num=nextq(),
                    )
                    ef_t = io_pool.tile([P, 4, BLK], BF16, tag="ef")
                    nc.sync.dma_start(out=ef_t[:], in_=eft[b])

                    msgT = msg_pool.tile([P, 4, BLK], BF16, tag="msg")
                    nc.vector.tensor_add(msgT[:], nf_t[:], ef_t[:])

                    h_t = h_pool.tile([P, 4, Fdim], F32, tag="h")
                    for t in range(4):
                        embT_ps = psT_pool.tile([P, D], BF16, tag="embT_ps")
                        for c in range(4):
                            nc.tensor.transpose(
                                out=embT_ps[:, c * P:(c + 1) * P],
                                in_=em_t[:, t, c * P:(c + 1) * P],
                                identity=ident_sb[:],
                            )
                        msgT2 = mt2_pool.tile([P, 4, P], BF16, tag="msgT2")
                        nc.vector.tensor_add(
                            msgT2[:], msgT[:, :, t * P:(t + 1) * P],
                            embT_ps[:].rearrange("p (c e) -> p c e", c=4))
                        out_ps = psO_pool.tile([P, Fdim], F32, tag="out_ps")
                        for c in range(4):
                            nc.tensor.matmul(
                                out_ps[:],
                                lhsT=msgT2[:, c, :],
                                rhs=wt_sb[:, c, :],
                                start=(c == 0), stop=(c == 3),
                            )
                        htmp = ht_pool.tile([P, Fdim], F32, tag="htmp")
                        nc.vector.tensor_add(htmp[:], out_ps[:], bias_sb[:])
                        nc.scalar.activation(
                            out=h_t[:, t, :], in_=htmp[:],
                            func=mybir.ActivationFunctionType.Relu,
                        )
                    nc.sync.dma_start(out=out[b], in_=h_t[:])
                    b += 1

    nc.compile()
    return nc


def plan_blocks(src_shards):
    nb = [0] * 4
    for s in src_shards:
        m = s % 4
        for k in range(4):
            g = int((m == k).sum())
            nb[k] = max(nb[k], (g + BLK - 1) // BLK)
    return nb


def pack_core(nb, nf_split, emb_bf, wt_bf, bias_full, ef_f32, src_ids, et_ids):
    """Pack one core's shard. Returns (in_map, slot_orig)."""
    NB = sum(nb)
    m = (src_ids % 4).astype(np.int64)

    slot_orig = np.full(NB * BLK, -1, dtype=np.int64)
    src_sorted = np.zeros(NB * BLK, dtype=np.int64)
    et_sorted = np.zeros(NB * BLK, dtype=np.int64)
    ef_sorted = np.zeros((NB * BLK, D), dtype=bf)
    pos = 0
    for k in range(4):
        idxs = np.nonzero(m == k)[0]
        n = len(idxs)
        slot_orig[pos:pos + n] = idxs
        src_sorted[pos:pos + n] = src_ids[idxs]
        src_sorted[pos + n:pos + nb[k] * BLK] = k  # pad rows: node k (k%4==k)
        et_sorted[pos:pos + n] = et_ids[idxs]
        ef_sorted[pos:pos + n] = ef_f32[idxs].astype(bf)
        pos += nb[k] * BLK

    idx16 = (src_sorted // 4).astype(np.int16)
    et_16 = et_sorted.astype(np.int16)

    def wrap16(a):  # [NB*BLK] -> [P, NB*32] wrapped-16, replicated x8
        w = a.reshape(NB, 32, 16)
        return np.ascontiguousarray(
            np.tile(w.transpose(2, 0, 1), (8, 1, 1)).reshape(P, NB * 32))

    eft = np.ascontiguousarray(
        ef_sorted.reshape(NB, BLK, 4, P).transpose(0, 3, 2, 1))

    in_map = {
        **{f"nf{k}": nf_split[k] for k in range(4)},
        "eft": eft,
        "emb": emb_bf,
        "src16": wrap16(idx16),
        "et16": wrap16(et_16),
        "wt": wt_bf,
        "biasf": bias_full,
    }
    return in_map, slot_orig


def unpack_core(out_arr, slot_orig, E):
    NB = out_arr.shape[0]
    h_sorted = out_arr.transpose(0, 2, 1, 3).reshape(NB * BLK, Fdim)
    h = np.zeros((E, Fdim), dtype=np.float32)
    valid = slot_orig >= 0
    h[slot_orig[valid]] = h_sorted[valid]
    return h


def prep_shared(node_feat, emb, W, b):
    nf_bf = node_feat.astype(bf)
    nf_split = [np.ascontiguousarray(nf_bf[k::4]) for k in range(4)]
    emb_bf = np.ascontiguousarray(emb.astype(bf))
    wt_bf = np.ascontiguousarray(W.T.astype(bf).reshape(4, P, Fdim))
    bias_full = np.ascontiguousarray(
        np.broadcast_to(b.astype(np.float32)[None, :], (P, Fdim)))
    return nf_split, emb_bf, wt_bf, bias_full


_module_cache = {}


def _get_module(nb):
    key = tuple(nb)
    if key not in _module_cache:
        _module_cache[key] = build_module(list(nb))
    return _module_cache[key]


def prepare(node_feat, edge_feat, src_idx, edge_type, emb, W, b):
    """Shard + pack all inputs. Returns (nc, in_maps, slot_origs, shard_bounds)."""
    node_feat = np.asarray(node_feat, dtype=np.float32)
    edge_feat = np.asarray(edge_feat, dtype=np.float32)
    src_idx = np.asarray(src_idx).astype(np.int64)
    edge_type = np.asarray(edge_type).astype(np.int64)
    emb = np.asarray(emb, dtype=np.float32)
    W = np.asarray(W, dtype=np.float32)
    b = np.asarray(b, dtype=np.float32)

    E = edge_feat.shape[0]
    E_shard = (E + N_CORES - 1) // N_CORES
    E_pad = E_shard * N_CORES
    if E_pad != E:
        pad = E_pad - E
        edge_feat = np.concatenate([edge_feat, np.zeros((pad, D), np.float32)])
        src_idx = np.concatenate([src_idx, np.zeros(pad, np.int64)])
        edge_type = np.concatenate([edge_type, np.zeros(pad, np.int64)])

    nf_split, emb_bf, wt_bf, bias_full = prep_shared(node_feat, emb, W, b)

    shards = [(c * E_shard, (c + 1) * E_shard) for c in range(N_CORES)]
    nb = plan_blocks([src_idx[a:z] for a, z in shards])
    nc = _get_module(nb)

    in_maps, slot_origs = [], []
    for a, z in shards:
        im, so = pack_core(nb, nf_split, emb_bf, wt_bf, bias_full,
                           edge_feat[a:z], src_idx[a:z], edge_type[a:z])
        in_maps.append(im)
        slot_origs.append(so)
    return nc, in_maps, slot_origs, shards, E, E_shard


def run(nc, in_maps, **kwargs):
    return bass_utils.run_bass_kernel_spmd(
        nc, in_maps, core_ids=list(range(N_CORES)), **kwargs)


def _sample_check(h, node_feat, edge_feat, src_idx, edge_type, emb, W, b, n=512):
    """Spot-check n random edges against a host bf16-precision recompute.
    Returns True if the device output is consistent (catches the rare
    intermittent SWDGE corruption, which is orders of magnitude larger
    than bf16 rounding)."""
    rng = np.random.default_rng(12345)
    E = h.shape[0]
    sel = rng.choice(E, size=min(n, E), replace=False)
    msg = (emb.astype(bf).astype(np.float32)[edge_type[sel]]
           + node_feat[src_idx[sel]].astype(bf).astype(np.float32)
           + edge_feat[sel].astype(bf).astype(np.float32))
    ref = np.maximum(msg @ W.T.astype(np.float32) + b.astype(np.float32), 0.0)
    scale = max(np.abs(ref).max(), 1.0)
    return np.abs(h[sel] - ref).max() <= 0.05 * scale


def kernel(node_feat, edge_feat, src_idx, edge_type, emb, W, b):
    node_feat = np.asarray(node_feat, dtype=np.float32)
    edge_feat = np.asarray(edge_feat, dtype=np.float32)
    src_idx = np.asarray(src_idx).astype(np.int64)
    edge_type = np.asarray(edge_type).astype(np.int64)
    emb = np.asarray(emb, dtype=np.float32)
    W = np.asarray(W, dtype=np.float32)
    b = np.asarray(b, dtype=np.float32)

    nc, in_maps, slot_origs, shards, E, E_shard = prepare(
        node_feat, edge_feat, src_idx, edge_type, emb, W, b)
    h = None
    for _attempt in range(4):
        res = run(nc, in_maps)
        outs = []
        for c in range(N_CORES):
            outs.append(unpack_core(res.results[c]["out"], slot_origs[c], E_shard))
        h = np.concatenate(outs, axis=0)[:E]
        if _sample_check(h, node_feat, edge_feat, src_idx, edge_type, emb, W, b):
            break
        # Corruption correlates with NEFF load state: force a reload before
        # retrying (the executable itself comes back from the compile cache).
        import jax
        jax.clear_caches()
    return h


# revision 9
# speedup vs baseline: 1.0474x; 1.0474x over previous
"""TRN2 Bass kernel for nn_AddMessagePasser (GNN message passing).

h = relu((emb[edge_type] + node_feat[src_idx] + edge_feat) @ W.T + b)

Strategy (8 NeuronCores, SPMD, data-parallel over edges):
- Edges padded to 8 equal shards; node_feat / emb / W / b replicated.
- Per core, edges are sorted by src%4 into 4 groups (padded to 512-edge
  blocks) so node-feature gathers can use int16 indices into the subtable
  node_feat[k::4] (25000 rows < 32768) via SWDGE dma_gather.
- All gathers use dma_gather(transpose=True): gathered rows land in
  transposed [d, e] layout [128, 4, 512] bf16, which feeds the matmul lhsT
  directly (no on-chip transposes). Gathers spread over 4 SWDGE queues.
- edge_feat is host-pre-transposed into the same layout; plain HWDGE DMA.
- msgT = nfT + efT + embT (2 DVE adds, bf16).
- Per 128-edge tile: 4 matmuls accumulate psum[128e, 512f] (bf16 x bf16,
  fp32 accumulate); DVE adds bias; ACT applies ReLU; one 1MB store/block.
"""
import numpy as np
import ml_dtypes

import concourse.bacc as bacc
import concourse.mybir as mybir
import concourse.tile as tile
from concourse import bass_utils

BF16 = mybir.dt.bfloat16
F32 = mybir.dt.float32
I16 = mybir.dt.int16

P = 128
D = 512
Fdim = 512
BLK = 512
N_NODES = 100000
N_RELS = 500
N_EDGES = 156250
N_CORES = 8
bf = ml_dtypes.bfloat16


def build_module(nb):
    """nb[k] = number of 512-edge blocks for src%4 group k (shared by all cores)."""
    NB = sum(nb)
    nc = bacc.Bacc("TRN2", debug=False, num_devices=1, num_swdge_queues=4)

    nfk = [nc.dram_tensor(f"nf{k}", [N_NODES // 4, D], BF16, kind="ExternalInput")
           for k in range(4)]
    eft = nc.dram_tensor("eft", [NB, P, 4, BLK], BF16, kind="ExternalInput")
    emb = nc.dram_tensor("emb", [N_RELS, D], BF16, kind="ExternalInput")
    src16 = nc.dram_tensor("src16", [P, NB * 32], I16, kind="ExternalInput")
    et16 = nc.dram_tensor("et16", [P, NB * 32], I16, kind="ExternalInput")
    wt = nc.dram_tensor("wt", [4, P, Fdim], BF16, kind="ExternalInput")
    biasf = nc.dram_tensor("biasf", [P, Fdim], F32, kind="ExternalInput")
    out = nc.dram_tensor("out", [NB, P, 4, Fdim], F32, kind="ExternalOutput")

    qctr = [0]

    def nextq():
        q = qctr[0] % 4
        qctr[0] += 1
        return q

    with tile.TileContext(nc) as tc:
        with (
            tc.tile_pool(name="const", bufs=1) as const_pool,
            tc.tile_pool(name="io", bufs=3) as io_pool,
            tc.tile_pool(name="msgp", bufs=4) as msg_pool,
            tc.tile_pool(name="hout", bufs=3) as h_pool,
            tc.tile_pool(name="htmp", bufs=6) as ht_pool,
            tc.tile_pool(name="pso", bufs=8, space="PSUM") as psO_pool,
        ):
            src_sb = const_pool.tile([P, NB * 32], I16)
            nc.sync.dma_start(out=src_sb[:], in_=src16[:])
            et_sb = const_pool.tile([P, NB * 32], I16)
            nc.sync.dma_start(out=et_sb[:], in_=et16[:])
            wt_sb = const_pool.tile([P, 4, Fdim], BF16)
            nc.scalar.dma_start(out=wt_sb[:], in_=wt[:].rearrange("c p f -> p c f"))
            bias_sb = const_pool.tile([P, Fdim], F32)
            nc.scalar.dma_start(out=bias_sb[:], in_=biasf[:])

            b = 0
            for k in range(4):
                for _ in range(nb[k]):
                    nf_t = io_pool.tile([P, 4, BLK], BF16, tag="nf")
                    nc.gpsimd.dma_gather(
                        out_ap=nf_t[:], in_ap=nfk[k][:],
                        idxs_ap=src_sb[:, b * 32:(b + 1) * 32],
                        num_idxs=BLK, num_idxs_reg=BLK, elem_size=D,
                        transpose=True, queue_num=nextq(),
                    )
                    em_t = io_pool.tile([P, 4, BLK], BF16, tag="em")
                    nc.gpsimd.dma_gather(
                        out_ap=em_t[:], in_ap=emb[:],
                        idxs_ap=et_sb[:, b * 32:(b + 1) * 32],
                        num_idxs=BLK, num_idxs_reg=BLK, elem_size=D,
                        transpose=True, queue_num=nextq(),
                    )
                    ef_t = io_pool.tile([P, 4, BLK], BF16, tag="ef")
                    nc.sync.dma_start(out=ef_t[:], in_=eft[b])

                    msgT = msg_pool.tile([P, 4, BLK], BF16, tag="msg")
                    nc.vector.tensor_add(msgT[:], nf_t[:], ef_t[:])
                    nc.vector.tensor_add(msgT[:], msgT[:], em_t[:])

                    h_t = h_pool.tile([P, 4, Fdim], F32, tag="h")
                    for t in range(4):
                        out_ps = psO_pool.tile([P, Fdim], F32, tag="out_ps")
                        for c in range(4):
                            nc.tensor.matmul(
                                out_ps[:],
                                lhsT=msgT[:, c, t * P:(t + 1) * P],
                                rhs=wt_sb[:, c, :],
                                start=(c == 0), stop=(c == 3),
                            )
                        htmp = ht_pool.tile([P, Fdim], F32, tag="htmp")
                        nc.vector.tensor_add(htmp[:], out_ps[:], bias_sb[:])
                        nc.scalar.activation(
                            out=h_t[:, t, :], in_=htmp[:],
                            func=mybir.ActivationFunctionType.Relu,
                        )
                    nc.sync.dma_start(out=out[b], in_=h_t[:])
                    b += 1

    nc.compile()
    return nc


def plan_blocks(src_shards):
    nb = [0] * 4
    for s in src_shards:
        m = s % 4
        for k in range(4):
            g = int((m == k).sum())
            nb[k] = max(nb[k], (g + BLK - 1) // BLK)
    return nb


def pack_core(nb, nf_split, emb_bf, wt_bf, bias_full, ef_f32, src_ids, et_ids):
    """Pack one core's shard. Returns (in_map, slot_orig)."""
    NB = sum(nb)
    m = (src_ids % 4).astype(np.int64)

    slot_orig = np.full(NB * BLK, -1, dtype=np.int64)
    src_sorted = np.zeros(NB * BLK, dtype=np.int64)
    et_sorted = np.zeros(NB * BLK, dtype=np.int64)
    ef_sorted = np.zeros((NB * BLK, D), dtype=bf)
    pos = 0
    for k in range(4):
        idxs = np.nonzero(m == k)[0]
        n = len(idxs)
        slot_orig[pos:pos + n] = idxs
        src_sorted[pos:pos + n] = src_ids[idxs]
        src_sorted[pos + n:pos + nb[k] * BLK] = k  # pad rows: node k (k%4==k)
        et_sorted[pos:pos + n] = et_ids[idxs]
        ef_sorted[pos:pos + n] = ef_f32[idxs].astype(bf)
        pos += nb[k] * BLK

    idx16 = (src_sorted // 4).astype(np.int16)
    et_16 = et_sorted.astype(np.int16)

    def wrap16(a):  # [NB*BLK] -> [P, NB*32] wrapped-16, replicated x8
        w = a.reshape(NB, 32, 16)
        return np.ascontiguousarray(
            np.tile(w.transpose(2, 0, 1), (8, 1, 1)).reshape(P, NB * 32))

    eft = np.ascontiguousarray(
        ef_sorted.reshape(NB, BLK, 4, P).transpose(0, 3, 2, 1))

    in_map = {
        **{f"nf{k}": nf_split[k] for k in range(4)},
        "eft": eft,
        "emb": emb_bf,
        "src16": wrap16(idx16),
        "et16": wrap16(et_16),
        "wt": wt_bf,
        "biasf": bias_full,
    }
    return in_map, slot_orig


def unpack_core(out_arr, slot_orig, E):
    NB = out_arr.shape[0]
    h_sorted = out_arr.transpose(0, 2, 1, 3).reshape(NB * BLK, Fdim)
    h = np.zeros((E, Fdim), dtype=np.float32)
    valid = slot_orig >= 0
    h[slot_orig[valid]] = h_sorted[valid]
    return h


def prep_shared(node_feat, emb, W, b):
    nf_bf = node_feat.astype(bf)
    nf_split = [np.ascontiguousarray(nf_bf[k::4]) for k in range(4)]
    emb_bf = np.ascontiguousarray(emb.astype(bf))
    wt_bf = np.ascontiguousarray(W.T.astype(bf).reshape(4, P, Fdim))
    bias_full = np.ascontiguousarray(
        np.broadcast_to(b.astype(np.float32)[None, :], (P, Fdim)))
    return nf_split, emb_bf, wt_bf, bias_full


_module_cache = {}


def _get_module(nb):
    key = tuple(nb)
    if key not in _module_cache:
        _module_cache[key] = build_module(list(nb))
    return _module_cache[key]


def prepare(node_feat, edge_feat, src_idx, edge_type, emb, W, b):
    """Shard + pack all inputs. Returns (nc, in_maps, slot_origs, shard_bounds)."""
    node_feat = np.asarray(node_feat, dtype=np.float32)
    edge_feat = np.asarray(edge_feat, dtype=np.float32)
    src_idx = np.asarray(src_idx).astype(np.int64)
    edge_type = np.asarray(edge_type).astype(np.int64)
    emb = np.asarray(emb, dtype=np.float32)
    W = np.asarray(W, dtype=np.float32)
    b = np.asarray(b, dtype=np.float32)

    E = edge_feat.shape[0]
    E_shard = (E + N_CORES - 1) // N_CORES
    E_pad = E_shard * N_CORES
    if E_pad != E:
        pad = E_pad - E
        edge_feat = np.concatenate([edge_feat, np.zeros((pad, D), np.float32)])
        src_idx = np.concatenate([src_idx, np.zeros(pad, np.int64)])
        edge_type = np.concatenate([edge_type, np.zeros(pad, np.int64)])

    nf_split, emb_bf, wt_bf, bias_full = prep_shared(node_feat, emb, W, b)

    shards = [(c * E_shard, (c + 1) * E_shard) for c in range(N_CORES)]
    nb = plan_blocks([src_idx[a:z] for a, z in shards])
    nc = _get_module(nb)

    in_maps, slot_origs = [], []
    for a, z in shards:
        im, so = pack_core(nb, nf_split, emb_bf, wt_bf, bias_full,
                           edge_feat[a:z], src_idx[a:z], edge_type[a:z])
        in_maps.append(im)
        slot_origs.append(so)
    return nc, in_maps, slot_origs, shards, E, E_shard


def run(nc, in_maps, **kwargs):
    return bass_utils.run_bass_kernel_spmd(
        nc, in_maps, core_ids=list(range(N_CORES)), **kwargs)


def _sample_check(h, node_feat, edge_feat, src_idx, edge_type, emb, W, b, n=512):
    """Spot-check n random edges against a host bf16-precision recompute.
    Returns True if the device output is consistent (catches the rare
    intermittent SWDGE corruption, which is orders of magnitude larger
    than bf16 rounding)."""
    rng = np.random.default_rng(12345)
    E = h.shape[0]
    sel = rng.choice(E, size=min(n, E), replace=False)
    msg = (emb.astype(bf).astype(np.float32)[edge_type[sel]]
           + node_feat[src_idx[sel]].astype(bf).astype(np.float32)
           + edge_feat[sel].astype(bf).astype(np.float32))
    ref = np.maximum(msg @ W.T.astype(np.float32) + b.astype(np.float32), 0.0)
    scale = max(np.abs(ref).max(), 1.0)
    return np.abs(h[sel] - ref).max() <= 0.05 * scale


def kernel(node_feat, edge_feat, src_idx, edge_type, emb, W, b):
    node_feat = np.asarray(node_feat, dtype=np.float32)
    edge_feat = np.asarray(edge_feat, dtype=np.float32)
    src_idx = np.asarray(src_idx).astype(np.int64)
    edge_type = np.asarray(edge_type).astype(np.int64)
    emb = np.asarray(emb, dtype=np.float32)
    W = np.asarray(W, dtype=np.float32)
    b = np.asarray(b, dtype=np.float32)

    nc, in_maps, slot_origs, shards, E, E_shard = prepare(
        node_feat, edge_feat, src_idx, edge_type, emb, W, b)
    h = None
    for _attempt in range(4):
        res = run(nc, in_maps)
        outs = []
        for c in range(N_CORES):
            outs.append(unpack_core(res.results[c]["out"], slot_origs[c], E_shard))
        h = np.concatenate(outs, axis=0)[:E]
        if _sample_check(h, node_feat, edge_feat, src_idx, edge_type, emb, W, b):
            break
        # Corruption correlates with NEFF load state: force a reload before
        # retrying (the executable itself comes back from the compile cache).
        import jax
        jax.clear_caches()
    return h
